# revision 4
# baseline (speedup 1.0000x reference)
"""GATv2Conv (heads=1, edge_dim=11, self-loops fill='mean') on 8 Trainium2 cores.

Dst-sharded (12500 own nodes/core). Per 128-dst chunk, segment reductions are
one-hot matmuls S^T @ [ex | e | ex*xl] accumulated in PSUM. The one-hot S is
GATHERED (fp8, exact 0/1) together with xr[dst] from a device-built dst-table
(rows [xr bf16 64 | onehot fp8 128] = 256B), and xl[src] is gathered from the
node table C[n] = [xl|xr] (bf16, 4 int16 row-banks). Gathers are batched per
~64-tile group (5 calls/group) to amortize the ~1us SWDGE fixed cost.

Self-loop attr term uses linearity: (sum attr)/deg @ W_e == (sum e)/deg, so e
is aggregated instead of attr and the epilogue needs no transposes.

Host work is layout only: sharding, sorting, index packing, dtype casts.
"""

import sys

sys.path.insert(0, "/opt/trn_rl_repo")

import numpy as np
import ml_dtypes

import concourse.bass as bass
import concourse.bacc as bacc
import concourse.tile as tile
import concourse.mybir as mybir
from concourse.bass_utils import run_bass_kernel_spmd

BF16 = ml_dtypes.bfloat16
AF = mybir.ActivationFunctionType
OP = mybir.AluOpType

N, E, DIN, DOUT, DE = 100000, 1000000, 128, 64, 11
NEG_SLOPE = 0.2
NC = 8
NOWN = N // NC                    # 12500
NCHUNK = 98                       # ceil(12500/128)
NPAD = NCHUNK * 128               # 12544
BANKS = 4
BANKROWS = 25088                  # 4*25088 = 100352 = XT_COLS; < 2^15 for int16
XT_COLS = BANKS * BANKROWS
DST_ROWS = NPAD + 1               # row 0 = zeros (pad target)
RHS_W = 129                       # [ex | e(64) | exxl(64)]
GROUP_TILES = 64                  # gather-group budget
GROUP_CHUNKS = 8                  # aggs slab depth
HGT = 32                          # half-group tiles (attr/rhs slab width)
SUB = 8                           # tiles per e-psum sub-block

last_exec_time_ns = None
last_insts = None
_CACHE = {}


def _cdiv(a, b):
    return -(-a // b)


def _bc3(ap2, mid):
    """[P, F] AP -> [P, mid, F] broadcast along a new middle dim."""
    return bass.AP(ap2.tensor, ap2.offset, [ap2.ap[0], [0, mid], ap2.ap[1]])


def _in3(ap2, inner):
    """[P, T] AP -> [P, T, inner] broadcast along a new inner dim."""
    return bass.AP(ap2.tensor, ap2.offset, [ap2.ap[0], ap2.ap[1], [0, inner]])


def _bcast_last(ap3, inner):
    """[P, T, 1] AP -> [P, T, inner] broadcast along the last dim."""
    return bass.AP(ap3.tensor, ap3.offset, [ap3.ap[0], ap3.ap[1], [0, inner]])


# --------------------------------------------------------------------------
# host-side layout (index manipulation only)
# --------------------------------------------------------------------------

def _plan(edge_index):
    src = np.asarray(edge_index[0]).astype(np.int64)
    dst = np.asarray(edge_index[1]).astype(np.int64)
    core = dst // NOWN
    ldst = dst - core * NOWN                      # 0..12499
    chunk = ldst >> 7                             # 0..97
    bank = src // BANKROWS                        # 0..3

    cellid = (core * NCHUNK + chunk) * BANKS + bank
    counts = np.bincount(cellid, minlength=NC * NCHUNK * BANKS)
    maxcnt = counts.reshape(NC, NCHUNK, BANKS).max(axis=0)     # [98, 4] shared
    ntiles_cb = (maxcnt + 127) // 128                          # [98, 4]
    t_ch = ntiles_cb.sum(axis=1)                               # [98]

    # group packing: consecutive chunks, sum(t_ch) <= GROUP_TILES, nch <= 8
    groups = []
    ch = 0
    while ch < NCHUNK:
        ch0, tl = ch, 0
        while (ch < NCHUNK and ch - ch0 < GROUP_CHUNKS
               and tl + t_ch[ch] <= GROUP_TILES):
            tl += int(t_ch[ch])
            ch += 1
        assert ch > ch0, f"chunk {ch0} too big: t_ch={t_ch[ch0]}"
        groups.append((ch0, ch - ch0, tl))

    # tile index assignment: group -> bank -> chunk
    cell_tile0 = np.zeros((NCHUNK, BANKS), np.int64)
    grp_meta = []
    tbase = 0
    for (ch0, nch, gt) in groups:
        g_tile0 = tbase
        bank_spans = []
        tile_chunk = []                    # chunk id per local tile
        for b in range(BANKS):
            b_t0 = tbase
            for c in range(ch0, ch0 + nch):
                cell_tile0[c, b] = tbase
                tbase += int(ntiles_cb[c, b])
                tile_chunk += [c] * int(ntiles_cb[c, b])
            bank_spans.append((b_t0 - g_tile0, tbase - b_t0))  # (local ofs, ntiles)
        grp_meta.append(dict(ch0=ch0, nch=nch, tile0=g_tile0, gt=gt,
                             bank_spans=bank_spans, tile_chunk=tile_chunk))
        assert gt == tbase - g_tile0
    tot_tiles = tbase

    # per-edge slot: stable sort by cell start, rank within cell
    cell_start = cell_tile0[chunk, bank] * 128       # global slot base per edge
    order = np.argsort(core * (tot_tiles * 128) + cell_start, kind="stable")
    key_s = cell_start[order]
    core_s = core[order]
    # rank within (core, cell)
    csort = core_s * (tot_tiles * 128) + key_s
    starts = np.zeros(E, np.int64)
    newgrp = np.ones(E, bool)
    newgrp[1:] = csort[1:] != csort[:-1]
    idx_of_start = np.nonzero(newgrp)[0]
    grp_id = np.cumsum(newgrp) - 1
    rank = np.arange(E) - idx_of_start[grp_id]
    slot = key_s + rank                              # global slot per edge

    return dict(src_o=src[order], ldst_o=ldst[order], core_o=core_s,
                slot=slot, bank_o=bank[order], order=order,
                maxcnt=maxcnt, ntiles_cb=ntiles_cb, t_ch=t_ch,
                groups=grp_meta, tot_tiles=tot_tiles)


def _host_arrays(plan, edge_attr, dst):
    tot_tiles = plan["tot_tiles"]
    idxcols = tot_tiles * 8
    attr_o = np.asarray(edge_attr)[plan["order"]]
    per_core = []
    for c in range(NC):
        m = plan["core_o"] == c
        slot = plan["slot"][m]
        p16, c16 = slot % 16, slot // 16

        i_src = np.zeros((16, idxcols), np.int16)
        i_src[p16, c16] = (plan["src_o"][m] - plan["bank_o"][m] * BANKROWS
                           ).astype(np.int16)
        i_dst = np.zeros((16, idxcols), np.int16)
        i_dst[p16, c16] = (plan["ldst_o"][m] + 1).astype(np.int16)

        attr_t = np.zeros((DE, tot_tiles * 128), BF16)
        attr_t[:, slot] = attr_o[m].T.astype(BF16)

        deg = np.bincount(dst[(dst // NOWN) == c] - c * NOWN, minlength=NPAD)
        rdeg = (1.0 / np.maximum(deg, 1)).astype(np.float32)

        per_core.append(dict(idx_src=np.tile(i_src, (8, 1)),
                             idx_dst=np.tile(i_dst, (8, 1)),
                             attr_t=attr_t,
                             rdeg=rdeg.reshape(NCHUNK, 128).T.copy()))
    return per_core


def _onehot_fp8():
    oh = np.zeros((DST_ROWS, 128), np.uint8)
    j = np.arange(1, DST_ROWS)
    oh[j, (j - 1) & 127] = 0x38                      # fp8 e4m3 1.0
    return oh.view(BF16)                             # [DST_ROWS, 64] bf16-viewed


# --------------------------------------------------------------------------
# device program (one SPMD NEFF for 8 cores; layout baked from `plan`)
# --------------------------------------------------------------------------

def _build_device(plan):
    dt = mybir.dt
    groups = plan["groups"]
    tot_tiles = plan["tot_tiles"]

    nc = bacc.Bacc("TRN2", target_bir_lowering=False, debug=False,
                   num_devices=NC, num_swdge_queues=4)

    xT = nc.dram_tensor("xT", [128, XT_COLS], dt.bfloat16, kind="ExternalInput")
    x_ownT = nc.dram_tensor("x_ownT", [128, NPAD], dt.bfloat16, kind="ExternalInput")
    w_cat = nc.dram_tensor("w_cat", [128, 128], dt.bfloat16, kind="ExternalInput")
    wep = nc.dram_tensor("wep", [128, DOUT], dt.bfloat16, kind="ExternalInput")
    att_bc = nc.dram_tensor("att_bc", [128, DOUT], dt.bfloat16, kind="ExternalInput")
    oneh = nc.dram_tensor("oneh", [DST_ROWS, DOUT], dt.bfloat16, kind="ExternalInput")
    rdeg_d = nc.dram_tensor("rdeg", [128, NCHUNK], dt.float32, kind="ExternalInput")
    idx_src = nc.dram_tensor("idx_src", [128, tot_tiles * 8], dt.int16, kind="ExternalInput")
    idx_dst = nc.dram_tensor("idx_dst", [128, tot_tiles * 8], dt.int16, kind="ExternalInput")
    attr_t = nc.dram_tensor("attr_t", [DE, tot_tiles * 128], dt.bfloat16, kind="ExternalInput")
    out_d = nc.dram_tensor("out", [NOWN, DOUT], dt.float32, kind="ExternalOutput")

    qn = [0]

    def next_q():
        q = qn[0] & 3
        qn[0] += 1
        return q

    with tile.TileContext(nc) as tc:
        with (
            tc.tile_pool(name="const", bufs=1) as constp,
            tc.tile_pool(name="own", bufs=1) as ownp,
            tc.tile_pool(name="dram", bufs=1, space="DRAM") as dramp,
            tc.tile_pool(name="tload", bufs=2) as tloadp,
            tc.tile_pool(name="stage", bufs=2) as stagep,
            tc.tile_pool(name="gat", bufs=2) as gatp,
            tc.tile_pool(name="attr", bufs=2) as attrp,
            tc.tile_pool(name="rhs", bufs=2) as rhsp,
            tc.tile_pool(name="work", bufs=3) as workp,
            tc.tile_pool(name="agg", bufs=2) as aggp,
            tc.tile_pool(name="epi", bufs=2) as epip,
            tc.tile_pool(name="idx", bufs=2) as idxp,
            tc.tile_pool(name="psC", bufs=2, space="PSUM") as psC,
            tc.tile_pool(name="psE", bufs=2, space="PSUM") as psE,
            tc.tile_pool(name="psA", bufs=2, space="PSUM") as psA,
        ):
            # ---------------- constants
            wcat_b = constp.tile([128, 128], dt.bfloat16, tag="wcat_b")
            nc.sync.dma_start(wcat_b[:], w_cat[:])
            wep_b = constp.tile([128, DOUT], dt.bfloat16, tag="wep_b")
            nc.sync.dma_start(wep_b[:], wep[:])
            attb_b = constp.tile([128, DOUT], dt.bfloat16, tag="attb_b")
            nc.sync.dma_start(attb_b[:], att_bc[:])
            rdeg_sb = constp.tile([128, NCHUNK], dt.float32, tag="rdeg_sb")
            nc.sync.dma_start(rdeg_sb[:], rdeg_d[:])

            xl_own = ownp.tile([128, NCHUNK, DOUT], dt.bfloat16, tag="xl_own")
            xr_own = ownp.tile([128, NCHUNK, DOUT], dt.bfloat16, tag="xr_own")

            C_b = [dramp.tile([BANKROWS, 128], dt.bfloat16, name=f"C_b{b}")
                   for b in range(BANKS)]
            dst_tab = dramp.tile([DST_ROWS, 128], dt.bfloat16)



            # ---------------- phase B: own nodes, dst-table
            zrow = constp.tile([1, 128], dt.bfloat16, tag="zrow")
            nc.vector.memset(zrow[:], 0.0)
            nc.sync.dma_start(dst_tab[0:1, :], zrow[:])
            nc.sync.dma_start(dst_tab[1:1 + NPAD, DOUT:128],
                              oneh[1:1 + NPAD, :])
            for q in range(25):                       # quads of 4 chunks (98 = 24*4+2)
                nq = 4 if q < 24 else 2
                ch0 = q * 4
                xo = tloadp.tile([128, 4, 128], dt.bfloat16, tag="xo")
                nc.sync.dma_start(
                    xo[:, 0:nq, :],
                    x_ownT[:, ch0 * 128:(ch0 + nq) * 128].rearrange(
                        "p (t c) -> p t c", c=128))
                ps = psC.tile([128, 512], dt.float32, tag="psC")
                for k in range(nq):
                    nc.tensor.matmul(ps[:, k * 128:(k + 1) * 128],
                                     lhsT=xo[:, k, :], rhs=wcat_b[:],
                                     start=True, stop=True)
                ps3 = ps[:, 0:nq * 128].rearrange("p (t c) -> p t c", c=128)
                nc.vector.tensor_copy(xl_own[:, ch0:ch0 + nq, :], ps3[:, :, 0:DOUT])
                nc.scalar.copy(xr_own[:, ch0:ch0 + nq, :], ps3[:, :, DOUT:128])
            nc.sync.dma_start(
                dst_tab[1:1 + NPAD, 0:DOUT].rearrange("(c p) d -> p c d", p=128),
                xr_own[:, :, :])

            # ---------------- phase A: node table C[n] = [xl | xr] per bank
            NT_IT = 14                                # tiles per iteration
            for b in range(BANKS):
                for it in range(BANKROWS // (NT_IT * 128)):   # 14 iters
                    c0 = b * BANKROWS + it * NT_IT * 128
                    xt = tloadp.tile([128, NT_IT, 128], dt.bfloat16, tag="xt")
                    nc.sync.dma_start(
                        xt[:, :, :],
                        xT[:, c0:c0 + NT_IT * 128].rearrange(
                            "p (t c) -> p t c", c=128))
                    stg = stagep.tile([128, NT_IT, 128], dt.bfloat16, tag="stg")
                    for q in range(NT_IT // 2):       # 7 psum pairs of 2 tiles
                        ps = psC.tile([128, 512], dt.float32, tag="psC")
                        for k in range(2):
                            nc.tensor.matmul(ps[:, k * 128:(k + 1) * 128],
                                             lhsT=xt[:, q * 2 + k, :],
                                             rhs=wcat_b[:], start=True, stop=True)
                        eng = nc.vector if (q & 1) else nc.scalar
                        if q & 1:
                            nc.vector.tensor_copy(
                                stg[:, q * 2:q * 2 + 2, :],
                                ps[:, 0:256].rearrange("p (t c) -> p t c", c=128))
                        else:
                            nc.scalar.copy(
                                stg[:, q * 2:q * 2 + 2, :],
                                ps[:, 0:256].rearrange("p (t c) -> p t c", c=128))
                    nc.sync.dma_start(
                        C_b[b][it * NT_IT * 128:(it + 1) * NT_IT * 128, :]
                        .rearrange("(t p) c -> p t c", p=128),
                        stg[:, :, :])

            # ---------------- phase C: per-group edge pipeline
            for g in groups:
                ch0, nch, t0, gt = g["ch0"], g["nch"], g["tile0"], g["gt"]
                tile_chunk = g["tile_chunk"]

                isrc = idxp.tile([128, GROUP_TILES * 8], dt.int16, tag="isrc")
                nc.sync.dma_start(isrc[:, 0:gt * 8],
                                  idx_src[:, t0 * 8:(t0 + gt) * 8])
                idst = idxp.tile([128, GROUP_TILES * 8], dt.int16, tag="idst")
                nc.sync.dma_start(idst[:, 0:gt * 8],
                                  idx_dst[:, t0 * 8:(t0 + gt) * 8])

                g_src = gatp.tile([128, GROUP_TILES, 128], dt.bfloat16, tag="g_src")
                for (lofs, nt) in g["bank_spans"]:
                    pass
                for b, (lofs, nt) in enumerate(g["bank_spans"]):
                    if nt == 0:
                        continue
                    nc.gpsimd.dma_gather(
                        out_ap=g_src[:, lofs:lofs + nt, :],
                        in_ap=C_b[b][:, :],
                        idxs_ap=isrc[:, lofs * 8:(lofs + nt) * 8],
                        num_idxs=nt * 128, num_idxs_reg=nt * 128,
                        elem_size=128, queue_num=0, single_packet=False)
                g_dst = gatp.tile([128, GROUP_TILES, 128], dt.bfloat16, tag="g_dst")
                nc.gpsimd.dma_gather(
                    out_ap=g_dst[:, 0:gt, :], in_ap=dst_tab[:, :],
                    idxs_ap=idst[:, 0:gt * 8],
                    num_idxs=gt * 128, num_idxs_reg=gt * 128,
                    elem_size=128, queue_num=0, single_packet=False)

                aggs = aggp.tile([128, GROUP_CHUNKS, RHS_W], dt.float32, tag="aggs")
                seen_chunk = set()

                nhalf = _cdiv(gt, HGT)
                for h in range(nhalf):
                    h0 = h * HGT
                    hn = min(HGT, gt - h0)
                    apad = attrp.tile([DE, HGT, 128], dt.bfloat16, tag="attrpad")
                    nc.sync.dma_start(
                        apad[:, 0:hn, :],
                        attr_t[:, (t0 + h0) * 128:(t0 + h0 + hn) * 128]
                        .rearrange("a (t c) -> a t c", c=128))
                    rhs = rhsp.tile([128, HGT, RHS_W], dt.bfloat16, tag="rhs")

                    for s0 in range(0, hn, SUB):
                        ns = min(SUB, hn - s0)
                        ts = h0 + s0                      # group-local tile base
                        pse = psE.tile([128, SUB * DOUT], dt.float32, tag="psE")
                        for i in range(ns):
                            nc.tensor.matmul(pse[:, i * DOUT:(i + 1) * DOUT],
                                             lhsT=apad[:, s0 + i, :],
                                             rhs=wep_b[0:DE, :],
                                             start=True, stop=True)
                        pse3 = pse[:, 0:ns * DOUT].rearrange("p (t d) -> p t d", d=DOUT)
                        nc.scalar.copy(rhs[:, s0:s0 + ns, 1:1 + DOUT], pse3)

                        m1 = workp.tile([128, SUB, DOUT], dt.bfloat16, tag="m1")
                        nc.vector.tensor_tensor(
                            out=m1[:, 0:ns, :], in0=g_src[:, ts:ts + ns, 0:DOUT],
                            in1=g_dst[:, ts:ts + ns, 0:DOUT], op=OP.add)
                        nc.vector.tensor_tensor(
                            out=m1[:, 0:ns, :], in0=m1[:, 0:ns, :],
                            in1=rhs[:, s0:s0 + ns, 1:1 + DOUT], op=OP.add)
                        nc.scalar.activation(m1[:, 0:ns, :], m1[:, 0:ns, :],
                                             AF.Prelu, alpha=NEG_SLOPE)
                        lt = workp.tile([128, SUB, DOUT], dt.bfloat16, tag="lt")
                        nc.vector.tensor_tensor(out=lt[:, 0:ns, :], in0=m1[:, 0:ns, :],
                                                in1=_bc3(attb_b[:, :], ns), op=OP.mult)
                        lg = workp.tile([128, SUB], dt.float32, tag="lg")
                        nc.vector.tensor_reduce(out=lg[:, 0:ns], in_=lt[:, 0:ns, :],
                                                axis=mybir.AxisListType.X, op=OP.add)
                        nc.scalar.activation(
                            rhs[:, s0:s0 + ns, 0:1],
                            lg[:, 0:ns].rearrange("p (t o) -> p t o", o=1), AF.Exp)
                        nc.vector.tensor_tensor(
                            out=rhs[:, s0:s0 + ns, 1 + DOUT:RHS_W],
                            in0=g_src[:, ts:ts + ns, 0:DOUT],
                            in1=_bcast_last(rhs[:, s0:s0 + ns, 0:1], DOUT),
                            op=OP.mult)

                        # agg spans (runs of equal chunk within this sub-block)
                        i = 0
                        while i < ns:
                            j = i
                            cch = tile_chunk[ts + i]
                            while j < ns and tile_chunk[ts + j] == cch:
                                j += 1
                            pa = psA.tile([128, RHS_W], dt.float32, tag="psA")
                            for t in range(i, j):
                                nc.tensor.matmul(
                                    pa[:],
                                    lhsT=g_dst[:, ts + t, DOUT:128].bitcast(dt.float8e4),
                                    rhs=rhs[:, s0 + t, :],
                                    start=(t == i), stop=(t == j - 1))
                            cl = cch - ch0
                            if cch in seen_chunk:
                                nc.vector.tensor_tensor(out=aggs[:, cl, :],
                                                        in0=aggs[:, cl, :],
                                                        in1=pa[:], op=OP.add)
                            else:
                                seen_chunk.add(cch)
                                nc.vector.tensor_copy(aggs[:, cl, :], pa[:])
                            i = j

                # ---------- per-group epilogue (self-loop + normalize + store)
                lep = epip.tile([128, GROUP_CHUNKS, DOUT], dt.float32, tag="lep")
                nc.vector.tensor_tensor(out=lep[:, 0:nch, :],
                                        in0=aggs[:, 0:nch, 1:1 + DOUT],
                                        in1=_in3(rdeg_sb[:, ch0:ch0 + nch], DOUT),
                                        op=OP.mult)
                nc.vector.tensor_tensor(out=lep[:, 0:nch, :], in0=lep[:, 0:nch, :],
                                        in1=xl_own[:, ch0:ch0 + nch, :], op=OP.add)
                nc.vector.tensor_tensor(out=lep[:, 0:nch, :], in0=lep[:, 0:nch, :],
                                        in1=xr_own[:, ch0:ch0 + nch, :], op=OP.add)
                mlb = epip.tile([128, GROUP_CHUNKS, DOUT], dt.bfloat16, tag="mlb")
                nc.scalar.activation(mlb[:, 0:nch, :], lep[:, 0:nch, :],
                                     AF.Prelu, alpha=NEG_SLOPE)
                nc.vector.tensor_tensor(out=mlb[:, 0:nch, :], in0=mlb[:, 0:nch, :],
                                        in1=_bc3(attb_b[:, :], nch), op=OP.mult)
                exl = epip.tile([128, GROUP_CHUNKS], dt.float32, tag="exl")
                nc.vector.tensor_reduce(out=exl[:, 0:nch], in_=mlb[:, 0:nch, :],
                                        axis=mybir.AxisListType.X, op=OP.add)
                nc.scalar.activation(exl[:, 0:nch], exl[:, 0:nch], AF.Exp)
                rden = epip.tile([128, GROUP_CHUNKS], dt.float32, tag="rden")
                nc.vector.tensor_tensor(out=rden[:, 0:nch], in0=aggs[:, 0:nch, 0],
                                        in1=exl[:, 0:nch], op=OP.add)
                nc.vector.reciprocal(rden[:, 0:nch], rden[:, 0:nch])
                o = epip.tile([128, GROUP_CHUNKS, DOUT], dt.float32, tag="o")
                nc.vector.tensor_tensor(out=o[:, 0:nch, :],
                                        in0=xl_own[:, ch0:ch0 + nch, :],
                                        in1=_in3(exl[:, 0:nch], DOUT), op=OP.mult)
                nc.vector.tensor_tensor(out=o[:, 0:nch, :], in0=o[:, 0:nch, :],
                                        in1=aggs[:, 0:nch, 1 + DOUT:RHS_W], op=OP.add)
                nc.vector.tensor_tensor(out=o[:, 0:nch, :], in0=o[:, 0:nch, :],
                                        in1=_in3(rden[:, 0:nch], DOUT), op=OP.mult)

                nfull = nch if (ch0 + nch) * 128 <= NOWN else nch - 1
                if nfull > 0:
                    nc.sync.dma_start(
                        out_d[ch0 * 128:(ch0 + nfull) * 128, :]
                        .rearrange("(c p) d -> p c d", p=128),
                        o[:, 0:nfull, :])
                if nfull < nch:
                    rows = NOWN - (ch0 + nfull) * 128
                    nc.sync.dma_start(
                        out_d[(ch0 + nfull) * 128:NOWN, :],
                        o[0:rows, nfull, :])

    nc.compile()
    return nc


# --------------------------------------------------------------------------
# entry point
# --------------------------------------------------------------------------

def _prep_inputs(x, edge_index, edge_attr, W_l, W_r, W_e, att, plan):
    per_core = _host_arrays(plan, np.asarray(edge_attr, np.float32),
                            np.asarray(edge_index[1]).astype(np.int64))

    x = np.asarray(x, np.float32)
    xT = np.zeros((128, XT_COLS), BF16)
    xT[:, :N] = x.T.astype(BF16)
    w_cat = np.concatenate([np.asarray(W_l, np.float32),
                            np.asarray(W_r, np.float32)], axis=1).astype(BF16)
    wep = np.zeros((128, DOUT), BF16)
    wep[:DE] = np.asarray(W_e, np.float32).astype(BF16)
    att_bc = np.tile(np.asarray(att, np.float32)[None, :], (128, 1)).astype(BF16)
    oneh = _onehot_fp8()

    in_maps = []
    for c in range(NC):
        x_ownT = np.zeros((128, NPAD), BF16)
        x_ownT[:, :NOWN] = x[c * NOWN:(c + 1) * NOWN].T.astype(BF16)
        pc = per_core[c]
        in_maps.append({
            "xT": xT, "x_ownT": x_ownT, "w_cat": w_cat, "wep": wep,
            "att_bc": att_bc, "oneh": oneh, "rdeg": pc["rdeg"],
            "idx_src": pc["idx_src"], "idx_dst": pc["idx_dst"],
            "attr_t": pc["attr_t"],
        })
    return in_maps


def kernel(x, edge_index, edge_attr, W_l, W_r, W_e, att):
    global last_exec_time_ns, last_insts

    plan = _plan(edge_index)
    in_maps = _prep_inputs(x, edge_index, edge_attr, W_l, W_r, W_e, att, plan)

    key = plan["maxcnt"].tobytes()
    if key not in _CACHE:
        _CACHE[key] = _build_device(plan)
    nc = _CACHE[key]

    try:
        res = run_bass_kernel_spmd(nc, in_maps, core_ids=list(range(NC)), trace=True)
        last_exec_time_ns = res.exec_time_ns
        last_insts = res.instructions_and_trace[0] if res.instructions_and_trace else None
    except Exception:
        res = run_bass_kernel_spmd(nc, in_maps, core_ids=list(range(NC)), trace=False)
        last_exec_time_ns = None
        last_insts = None

    return np.concatenate([res.results[c]["out"] for c in range(NC)], axis=0)


# revision 7
# speedup vs baseline: 1.5112x; 1.5112x over previous
"""GATv2Conv (heads=1, edge_dim=11, self-loops fill='mean') on 8 Trainium2 cores.

Dst-sharded (12500 own nodes/core). Per 128-dst chunk, segment reductions are
one-hot matmuls S^T @ [ex | e | ex*xl] accumulated in PSUM. The one-hot S is
GATHERED (fp8, exact 0/1) together with xr[dst] from a device-built dst-table
(rows [xr bf16 64 | onehot fp8 128] = 256B), and xl[src] is gathered from the
node table C[n] = [xl|xr] (bf16, 4 int16 row-banks). Gathers are batched per
~64-tile group (5 calls/group) to amortize the ~1us SWDGE fixed cost.

Self-loop attr term uses linearity: (sum attr)/deg @ W_e == (sum e)/deg, so e
is aggregated instead of attr and the epilogue needs no transposes.

Host work is layout only: sharding, sorting, index packing, dtype casts.
"""

import sys

sys.path.insert(0, "/opt/trn_rl_repo")

import numpy as np
import ml_dtypes

import concourse.bass as bass
import concourse.bacc as bacc
import concourse.tile as tile
import concourse.mybir as mybir
from concourse.bass_utils import run_bass_kernel_spmd

BF16 = ml_dtypes.bfloat16
AF = mybir.ActivationFunctionType
OP = mybir.AluOpType

N, E, DIN, DOUT, DE = 100000, 1000000, 128, 64, 11
NEG_SLOPE = 0.2
NC = 8
NOWN = N // NC                    # 12500
NCHUNK = 98                       # ceil(12500/128)
NPAD = NCHUNK * 128               # 12544
BANKS = 4
BANKROWS = 25088                  # 4*25088 = 100352 = XT_COLS; < 2^15 for int16
XT_COLS = BANKS * BANKROWS
DST_ROWS = NPAD + 1               # row 0 = zeros (pad target)
RHS_W = 129                       # [ex | e(64) | exxl(64)]
GROUP_TILES = 64                  # gather-group budget
GROUP_CHUNKS = 8                  # aggs slab depth
HGT = 32                          # half-group tiles (attr/rhs slab width)
SUB = 8                           # tiles per e-psum sub-block

last_exec_time_ns = None
last_insts = None
_CACHE = {}


def _cdiv(a, b):
    return -(-a // b)


def _bc3(ap2, mid):
    """[P, F] AP -> [P, mid, F] broadcast along a new middle dim."""
    return bass.AP(ap2.tensor, ap2.offset, [ap2.ap[0], [0, mid], ap2.ap[1]])


def _in3(ap2, inner):
    """[P, T] AP -> [P, T, inner] broadcast along a new inner dim."""
    return bass.AP(ap2.tensor, ap2.offset, [ap2.ap[0], ap2.ap[1], [0, inner]])


def _bcast_last(ap3, inner):
    """[P, T, 1] AP -> [P, T, inner] broadcast along the last dim."""
    return bass.AP(ap3.tensor, ap3.offset, [ap3.ap[0], ap3.ap[1], [0, inner]])


# --------------------------------------------------------------------------
# host-side layout (index manipulation only)
# --------------------------------------------------------------------------

def _plan(edge_index):
    src = np.asarray(edge_index[0]).astype(np.int64)
    dst = np.asarray(edge_index[1]).astype(np.int64)
    core = dst // NOWN
    ldst = dst - core * NOWN                      # 0..12499
    chunk = ldst >> 7                             # 0..97
    bank = src // BANKROWS                        # 0..3

    cellid = (core * NCHUNK + chunk) * BANKS + bank
    counts = np.bincount(cellid, minlength=NC * NCHUNK * BANKS)
    maxcnt = counts.reshape(NC, NCHUNK, BANKS).max(axis=0)     # [98, 4] shared
    ntiles_cb = (maxcnt + 127) // 128                          # [98, 4]
    t_ch = ntiles_cb.sum(axis=1)                               # [98]

    # group packing: consecutive chunks, sum(t_ch) <= GROUP_TILES, nch <= 8
    groups = []
    ch = 0
    while ch < NCHUNK:
        ch0, tl = ch, 0
        while (ch < NCHUNK and ch - ch0 < GROUP_CHUNKS
               and tl + t_ch[ch] <= GROUP_TILES):
            tl += int(t_ch[ch])
            ch += 1
        assert ch > ch0, f"chunk {ch0} too big: t_ch={t_ch[ch0]}"
        groups.append((ch0, ch - ch0, tl))

    # tile index assignment: group -> bank -> chunk
    cell_tile0 = np.zeros((NCHUNK, BANKS), np.int64)
    grp_meta = []
    tbase = 0
    for (ch0, nch, gt) in groups:
        g_tile0 = tbase
        bank_spans = []
        tile_chunk = []                    # chunk id per local tile
        for b in range(BANKS):
            b_t0 = tbase
            for c in range(ch0, ch0 + nch):
                cell_tile0[c, b] = tbase
                tbase += int(ntiles_cb[c, b])
                tile_chunk += [c] * int(ntiles_cb[c, b])
            bank_spans.append((b_t0 - g_tile0, tbase - b_t0))  # (local ofs, ntiles)
        grp_meta.append(dict(ch0=ch0, nch=nch, tile0=g_tile0, gt=gt,
                             bank_spans=bank_spans, tile_chunk=tile_chunk))
        assert gt == tbase - g_tile0
    tot_tiles = tbase

    # per-edge slot: stable sort by cell start, rank within cell
    cell_start = cell_tile0[chunk, bank] * 128       # global slot base per edge
    order = np.argsort(core * (tot_tiles * 128) + cell_start, kind="stable")
    key_s = cell_start[order]
    core_s = core[order]
    # rank within (core, cell)
    csort = core_s * (tot_tiles * 128) + key_s
    starts = np.zeros(E, np.int64)
    newgrp = np.ones(E, bool)
    newgrp[1:] = csort[1:] != csort[:-1]
    idx_of_start = np.nonzero(newgrp)[0]
    grp_id = np.cumsum(newgrp) - 1
    rank = np.arange(E) - idx_of_start[grp_id]
    slot = key_s + rank                              # global slot per edge

    return dict(src_o=src[order], ldst_o=ldst[order], core_o=core_s,
                slot=slot, bank_o=bank[order], order=order,
                maxcnt=maxcnt, ntiles_cb=ntiles_cb, t_ch=t_ch,
                groups=grp_meta, tot_tiles=tot_tiles)


def _host_arrays(plan, edge_attr, dst):
    tot_tiles = plan["tot_tiles"]
    idxcols = tot_tiles * 8
    attr_o = np.asarray(edge_attr)[plan["order"]]
    per_core = []
    for c in range(NC):
        m = plan["core_o"] == c
        slot = plan["slot"][m]
        p16, c16 = slot % 16, slot // 16

        i_src = np.zeros((16, idxcols), np.int16)
        i_src[p16, c16] = (plan["src_o"][m] - plan["bank_o"][m] * BANKROWS
                           ).astype(np.int16)
        i_dst = np.zeros((16, idxcols), np.int16)
        i_dst[p16, c16] = (plan["ldst_o"][m] + 1).astype(np.int16)

        attr_t = np.zeros((DE, tot_tiles * 128), BF16)
        attr_t[:, slot] = attr_o[m].T.astype(BF16)

        deg = np.bincount(dst[(dst // NOWN) == c] - c * NOWN, minlength=NPAD)
        rdeg = (1.0 / np.maximum(deg, 1)).astype(np.float32)

        per_core.append(dict(idx_src=np.tile(i_src, (8, 1)),
                             idx_dst=np.tile(i_dst, (8, 1)),
                             attr_t=attr_t,
                             rdeg=rdeg.reshape(NCHUNK, 128).T.copy()))
    return per_core


def _onehot_fp8():
    oh = np.zeros((DST_ROWS, 128), np.uint8)
    j = np.arange(1, DST_ROWS)
    oh[j, (j - 1) & 127] = 0x38                      # fp8 e4m3 1.0
    return oh.view(BF16)                             # [DST_ROWS, 64] bf16-viewed


# --------------------------------------------------------------------------
# device program (one SPMD NEFF for 8 cores; layout baked from `plan`)
# --------------------------------------------------------------------------

def _build_device(plan, queue_map=None):
    """queue_map: list of queue_num per gather (emission order), or None (all 0).

    Tile assigns SWDGE completion-sem lanes (DMASW0-7) round-robin in
    SCHEDULED order; a lane must only ever be updated from one SWDGE queue
    or completion waits under-synchronize (a real HW race). So pass 1 builds
    with all gathers on queue 0, reads each gather's assigned lane, and pass 2
    rebuilds with queue = lane % 4 to get 4-way parallel descriptor gen.
    """
    dt = mybir.dt
    groups = plan["groups"]
    tot_tiles = plan["tot_tiles"]
    gathers = []

    nc = bacc.Bacc("TRN2", target_bir_lowering=False, debug=False,
                   num_devices=NC, num_swdge_queues=4)

    def gq():
        if queue_map is None:
            return 0
        return queue_map[len(gathers)]

    xT = nc.dram_tensor("xT", [128, XT_COLS], dt.bfloat16, kind="ExternalInput")
    x_ownT = nc.dram_tensor("x_ownT", [128, NPAD], dt.bfloat16, kind="ExternalInput")
    w_cat = nc.dram_tensor("w_cat", [128, 128], dt.bfloat16, kind="ExternalInput")
    wep = nc.dram_tensor("wep", [128, DOUT], dt.bfloat16, kind="ExternalInput")
    att_bc = nc.dram_tensor("att_bc", [128, DOUT], dt.bfloat16, kind="ExternalInput")
    oneh = nc.dram_tensor("oneh", [DST_ROWS, DOUT], dt.bfloat16, kind="ExternalInput")
    rdeg_d = nc.dram_tensor("rdeg", [128, NCHUNK], dt.float32, kind="ExternalInput")
    idx_src = nc.dram_tensor("idx_src", [128, tot_tiles * 8], dt.int16, kind="ExternalInput")
    idx_dst = nc.dram_tensor("idx_dst", [128, tot_tiles * 8], dt.int16, kind="ExternalInput")
    attr_t = nc.dram_tensor("attr_t", [DE, tot_tiles * 128], dt.bfloat16, kind="ExternalInput")
    out_d = nc.dram_tensor("out", [NOWN, DOUT], dt.float32, kind="ExternalOutput")

    qn = [0]

    def next_q():
        q = qn[0] & 3
        qn[0] += 1
        return q

    with tile.TileContext(nc) as tc:
        with (
            tc.tile_pool(name="const", bufs=1) as constp,
            tc.tile_pool(name="own", bufs=1) as ownp,
            tc.tile_pool(name="dram", bufs=1, space="DRAM") as dramp,
            tc.tile_pool(name="tload", bufs=2) as tloadp,
            tc.tile_pool(name="stage", bufs=2) as stagep,
            tc.tile_pool(name="gat", bufs=2) as gatp,
            tc.tile_pool(name="attr", bufs=2) as attrp,
            tc.tile_pool(name="rhs", bufs=2) as rhsp,
            tc.tile_pool(name="work", bufs=3) as workp,
            tc.tile_pool(name="agg", bufs=2) as aggp,
            tc.tile_pool(name="epi", bufs=2) as epip,
            tc.tile_pool(name="idx", bufs=2) as idxp,
            tc.tile_pool(name="psC", bufs=2, space="PSUM") as psC,
            tc.tile_pool(name="psE", bufs=2, space="PSUM") as psE,
            tc.tile_pool(name="psA", bufs=2, space="PSUM") as psA,
        ):
            # ---------------- constants
            wcat_b = constp.tile([128, 128], dt.bfloat16, tag="wcat_b")
            nc.sync.dma_start(wcat_b[:], w_cat[:])
            wep_b = constp.tile([128, DOUT], dt.bfloat16, tag="wep_b")
            nc.sync.dma_start(wep_b[:], wep[:])
            attb_b = constp.tile([128, DOUT], dt.bfloat16, tag="attb_b")
            nc.sync.dma_start(attb_b[:], att_bc[:])
            rdeg_sb = constp.tile([128, NCHUNK], dt.float32, tag="rdeg_sb")
            nc.sync.dma_start(rdeg_sb[:], rdeg_d[:])

            xl_own = ownp.tile([128, NCHUNK, DOUT], dt.bfloat16, tag="xl_own")
            xr_own = ownp.tile([128, NCHUNK, DOUT], dt.bfloat16, tag="xr_own")

            C_b = [dramp.tile([BANKROWS, 128], dt.bfloat16, name=f"C_b{b}")
                   for b in range(BANKS)]
            dst_tab = dramp.tile([DST_ROWS, 128], dt.bfloat16)



            # ---------------- phase B: own nodes, dst-table
            zrow = constp.tile([1, 128], dt.bfloat16, tag="zrow")
            nc.vector.memset(zrow[:], 0.0)
            nc.sync.dma_start(dst_tab[0:1, :], zrow[:])
            nc.sync.dma_start(dst_tab[1:1 + NPAD, DOUT:128],
                              oneh[1:1 + NPAD, :])
            for q in range(25):                       # quads of 4 chunks (98 = 24*4+2)
                nq = 4 if q < 24 else 2
                ch0 = q * 4
                xo = tloadp.tile([128, 4, 128], dt.bfloat16, tag="xo")
                nc.sync.dma_start(
                    xo[:, 0:nq, :],
                    x_ownT[:, ch0 * 128:(ch0 + nq) * 128].rearrange(
                        "p (t c) -> p t c", c=128))
                ps = psC.tile([128, 512], dt.float32, tag="psC")
                for k in range(nq):
                    nc.tensor.matmul(ps[:, k * 128:(k + 1) * 128],
                                     lhsT=xo[:, k, :], rhs=wcat_b[:],
                                     start=True, stop=True)
                ps3 = ps[:, 0:nq * 128].rearrange("p (t c) -> p t c", c=128)
                nc.vector.tensor_copy(xl_own[:, ch0:ch0 + nq, :], ps3[:, :, 0:DOUT])
                nc.scalar.copy(xr_own[:, ch0:ch0 + nq, :], ps3[:, :, DOUT:128])
            nc.sync.dma_start(
                dst_tab[1:1 + NPAD, 0:DOUT].rearrange("(c p) d -> p c d", p=128),
                xr_own[:, :, :])

            # ---------------- phase A: node table C[n] = [xl | xr] per bank
            NT_IT = 14                                # tiles per iteration
            for b in range(BANKS):
                for it in range(BANKROWS // (NT_IT * 128)):   # 14 iters
                    c0 = b * BANKROWS + it * NT_IT * 128
                    xt = tloadp.tile([128, NT_IT, 128], dt.bfloat16, tag="xt")
                    nc.sync.dma_start(
                        xt[:, :, :],
                        xT[:, c0:c0 + NT_IT * 128].rearrange(
                            "p (t c) -> p t c", c=128))
                    stg = stagep.tile([128, NT_IT, 128], dt.bfloat16, tag="stg")
                    for q in range(NT_IT // 2):       # 7 psum pairs of 2 tiles
                        ps = psC.tile([128, 512], dt.float32, tag="psC")
                        for k in range(2):
                            nc.tensor.matmul(ps[:, k * 128:(k + 1) * 128],
                                             lhsT=xt[:, q * 2 + k, :],
                                             rhs=wcat_b[:], start=True, stop=True)
                        eng = nc.vector if (q & 1) else nc.scalar
                        if q & 1:
                            nc.vector.tensor_copy(
                                stg[:, q * 2:q * 2 + 2, :],
                                ps[:, 0:256].rearrange("p (t c) -> p t c", c=128))
                        else:
                            nc.scalar.copy(
                                stg[:, q * 2:q * 2 + 2, :],
                                ps[:, 0:256].rearrange("p (t c) -> p t c", c=128))
                    nc.sync.dma_start(
                        C_b[b][it * NT_IT * 128:(it + 1) * NT_IT * 128, :]
                        .rearrange("(t p) c -> p t c", p=128),
                        stg[:, :, :])

            # ---------------- phase C: per-group edge pipeline
            for g in groups:
                ch0, nch, t0, gt = g["ch0"], g["nch"], g["tile0"], g["gt"]
                tile_chunk = g["tile_chunk"]

                isrc = idxp.tile([128, GROUP_TILES * 8], dt.int16, tag="isrc")
                nc.sync.dma_start(isrc[:, 0:gt * 8],
                                  idx_src[:, t0 * 8:(t0 + gt) * 8])
                idst = idxp.tile([128, GROUP_TILES * 8], dt.int16, tag="idst")
                nc.sync.dma_start(idst[:, 0:gt * 8],
                                  idx_dst[:, t0 * 8:(t0 + gt) * 8])

                g_src = gatp.tile([128, GROUP_TILES, 128], dt.bfloat16, tag="g_src")
                for b, (lofs, nt) in enumerate(g["bank_spans"]):
                    if nt == 0:
                        continue
                    gi = nc.gpsimd.dma_gather(
                        out_ap=g_src[:, lofs:lofs + nt, :],
                        in_ap=C_b[b][:, :],
                        idxs_ap=isrc[:, lofs * 8:(lofs + nt) * 8],
                        num_idxs=nt * 128, num_idxs_reg=nt * 128,
                        elem_size=128, queue_num=gq(), single_packet=False)
                    gathers.append(gi)
                g_dst = gatp.tile([128, GROUP_TILES, 128], dt.bfloat16, tag="g_dst")
                gi = nc.gpsimd.dma_gather(
                    out_ap=g_dst[:, 0:gt, :], in_ap=dst_tab[:, :],
                    idxs_ap=idst[:, 0:gt * 8],
                    num_idxs=gt * 128, num_idxs_reg=gt * 128,
                    elem_size=128, queue_num=gq(), single_packet=False)
                gathers.append(gi)

                aggs = aggp.tile([128, GROUP_CHUNKS, RHS_W], dt.float32, tag="aggs")
                seen_chunk = set()

                nhalf = _cdiv(gt, HGT)
                for h in range(nhalf):
                    h0 = h * HGT
                    hn = min(HGT, gt - h0)
                    apad = attrp.tile([DE, HGT, 128], dt.bfloat16, tag="attrpad")
                    nc.sync.dma_start(
                        apad[:, 0:hn, :],
                        attr_t[:, (t0 + h0) * 128:(t0 + h0 + hn) * 128]
                        .rearrange("a (t c) -> a t c", c=128))
                    rhs = rhsp.tile([128, HGT, RHS_W], dt.bfloat16, tag="rhs")

                    for s0 in range(0, hn, SUB):
                        ns = min(SUB, hn - s0)
                        ts = h0 + s0                      # group-local tile base
                        pse = psE.tile([128, SUB * DOUT], dt.float32, tag="psE")
                        for i in range(ns):
                            nc.tensor.matmul(pse[:, i * DOUT:(i + 1) * DOUT],
                                             lhsT=apad[:, s0 + i, :],
                                             rhs=wep_b[0:DE, :],
                                             start=True, stop=True)
                        pse3 = pse[:, 0:ns * DOUT].rearrange("p (t d) -> p t d", d=DOUT)
                        nc.scalar.copy(rhs[:, s0:s0 + ns, 1:1 + DOUT], pse3)

                        m1 = workp.tile([128, SUB, DOUT], dt.bfloat16, tag="m1")
                        nc.vector.tensor_tensor(
                            out=m1[:, 0:ns, :], in0=g_src[:, ts:ts + ns, 0:DOUT],
                            in1=g_dst[:, ts:ts + ns, 0:DOUT], op=OP.add)
                        nc.vector.tensor_tensor(
                            out=m1[:, 0:ns, :], in0=m1[:, 0:ns, :],
                            in1=rhs[:, s0:s0 + ns, 1:1 + DOUT], op=OP.add)
                        nc.scalar.activation(m1[:, 0:ns, :], m1[:, 0:ns, :],
                                             AF.Prelu, alpha=NEG_SLOPE)
                        lt = workp.tile([128, SUB, DOUT], dt.bfloat16, tag="lt")
                        nc.vector.tensor_tensor(out=lt[:, 0:ns, :], in0=m1[:, 0:ns, :],
                                                in1=_bc3(attb_b[:, :], ns), op=OP.mult)
                        lg = workp.tile([128, SUB], dt.float32, tag="lg")
                        nc.vector.tensor_reduce(out=lg[:, 0:ns], in_=lt[:, 0:ns, :],
                                                axis=mybir.AxisListType.X, op=OP.add)
                        nc.scalar.activation(
                            rhs[:, s0:s0 + ns, 0:1],
                            lg[:, 0:ns].rearrange("p (t o) -> p t o", o=1), AF.Exp)
                        nc.vector.tensor_tensor(
                            out=rhs[:, s0:s0 + ns, 1 + DOUT:RHS_W],
                            in0=g_src[:, ts:ts + ns, 0:DOUT],
                            in1=_bcast_last(rhs[:, s0:s0 + ns, 0:1], DOUT),
                            op=OP.mult)

                        # agg spans (runs of equal chunk within this sub-block)
                        i = 0
                        while i < ns:
                            j = i
                            cch = tile_chunk[ts + i]
                            while j < ns and tile_chunk[ts + j] == cch:
                                j += 1
                            pa = psA.tile([128, RHS_W], dt.float32, tag="psA")
                            for t in range(i, j):
                                nc.tensor.matmul(
                                    pa[:],
                                    lhsT=g_dst[:, ts + t, DOUT:128].bitcast(dt.float8e4),
                                    rhs=rhs[:, s0 + t, :],
                                    start=(t == i), stop=(t == j - 1))
                            cl = cch - ch0
                            if cch in seen_chunk:
                                nc.vector.tensor_tensor(out=aggs[:, cl, :],
                                                        in0=aggs[:, cl, :],
                                                        in1=pa[:], op=OP.add)
                            else:
                                seen_chunk.add(cch)
                                nc.vector.tensor_copy(aggs[:, cl, :], pa[:])
                            i = j

                # ---------- per-group epilogue (self-loop + normalize + store)
                lep = epip.tile([128, GROUP_CHUNKS, DOUT], dt.float32, tag="lep")
                nc.vector.tensor_tensor(out=lep[:, 0:nch, :],
                                        in0=aggs[:, 0:nch, 1:1 + DOUT],
                                        in1=_in3(rdeg_sb[:, ch0:ch0 + nch], DOUT),
                                        op=OP.mult)
                nc.vector.tensor_tensor(out=lep[:, 0:nch, :], in0=lep[:, 0:nch, :],
                                        in1=xl_own[:, ch0:ch0 + nch, :], op=OP.add)
                nc.vector.tensor_tensor(out=lep[:, 0:nch, :], in0=lep[:, 0:nch, :],
                                        in1=xr_own[:, ch0:ch0 + nch, :], op=OP.add)
                mlb = epip.tile([128, GROUP_CHUNKS, DOUT], dt.bfloat16, tag="mlb")
                nc.scalar.activation(mlb[:, 0:nch, :], lep[:, 0:nch, :],
                                     AF.Prelu, alpha=NEG_SLOPE)
                nc.vector.tensor_tensor(out=mlb[:, 0:nch, :], in0=mlb[:, 0:nch, :],
                                        in1=_bc3(attb_b[:, :], nch), op=OP.mult)
                exl = epip.tile([128, GROUP_CHUNKS], dt.float32, tag="exl")
                nc.vector.tensor_reduce(out=exl[:, 0:nch], in_=mlb[:, 0:nch, :],
                                        axis=mybir.AxisListType.X, op=OP.add)
                nc.scalar.activation(exl[:, 0:nch], exl[:, 0:nch], AF.Exp)
                rden = epip.tile([128, GROUP_CHUNKS], dt.float32, tag="rden")
                nc.vector.tensor_tensor(out=rden[:, 0:nch], in0=aggs[:, 0:nch, 0],
                                        in1=exl[:, 0:nch], op=OP.add)
                nc.vector.reciprocal(rden[:, 0:nch], rden[:, 0:nch])
                o = epip.tile([128, GROUP_CHUNKS, DOUT], dt.float32, tag="o")
                nc.vector.tensor_tensor(out=o[:, 0:nch, :],
                                        in0=xl_own[:, ch0:ch0 + nch, :],
                                        in1=_in3(exl[:, 0:nch], DOUT), op=OP.mult)
                nc.vector.tensor_tensor(out=o[:, 0:nch, :], in0=o[:, 0:nch, :],
                                        in1=aggs[:, 0:nch, 1 + DOUT:RHS_W], op=OP.add)
                nc.vector.tensor_tensor(out=o[:, 0:nch, :], in0=o[:, 0:nch, :],
                                        in1=_in3(rden[:, 0:nch], DOUT), op=OP.mult)

                nfull = nch if (ch0 + nch) * 128 <= NOWN else nch - 1
                if nfull > 0:
                    nc.sync.dma_start(
                        out_d[ch0 * 128:(ch0 + nfull) * 128, :]
                        .rearrange("(c p) d -> p c d", p=128),
                        o[:, 0:nfull, :])
                if nfull < nch:
                    rows = NOWN - (ch0 + nfull) * 128
                    nc.sync.dma_start(
                        out_d[(ch0 + nfull) * 128:NOWN, :],
                        o[0:rows, nfull, :])

    nc.compile()
    return nc, gathers


def _gather_lanes(gathers):
    """Read the scheduler-assigned DMASW lane for each gather."""
    from concourse.tile_scheduler import PROC_NAMES
    lanes = []
    for gi in gathers:
        proc = getattr(gi.ins, "bass_scheduled_proc", None)
        name = PROC_NAMES[proc] if proc is not None else None
        assert name is not None and name.startswith("DMASW"), (proc, name)
        lanes.append(int(name[5:]))
    return lanes


def _build_two_pass(plan):
    nc1, gathers1 = _build_device(plan)
    lanes = _gather_lanes(gathers1)
    qmap = [ln % 4 for ln in lanes]
    nc2, gathers2 = _build_device(plan, queue_map=qmap)
    lanes2 = _gather_lanes(gathers2)
    if lanes2 != lanes:                       # schedule shifted: fall back safe
        return nc1
    return nc2


# --------------------------------------------------------------------------
# entry point
# --------------------------------------------------------------------------

def _prep_inputs(x, edge_index, edge_attr, W_l, W_r, W_e, att, plan):
    per_core = _host_arrays(plan, np.asarray(edge_attr, np.float32),
                            np.asarray(edge_index[1]).astype(np.int64))

    x = np.asarray(x, np.float32)
    xT = np.zeros((128, XT_COLS), BF16)
    xT[:, :N] = x.T.astype(BF16)
    w_cat = np.concatenate([np.asarray(W_l, np.float32),
                            np.asarray(W_r, np.float32)], axis=1).astype(BF16)
    wep = np.zeros((128, DOUT), BF16)
    wep[:DE] = np.asarray(W_e, np.float32).astype(BF16)
    att_bc = np.tile(np.asarray(att, np.float32)[None, :], (128, 1)).astype(BF16)
    oneh = _onehot_fp8()

    in_maps = []
    for c in range(NC):
        x_ownT = np.zeros((128, NPAD), BF16)
        x_ownT[:, :NOWN] = x[c * NOWN:(c + 1) * NOWN].T.astype(BF16)
        pc = per_core[c]
        in_maps.append({
            "xT": xT, "x_ownT": x_ownT, "w_cat": w_cat, "wep": wep,
            "att_bc": att_bc, "oneh": oneh, "rdeg": pc["rdeg"],
            "idx_src": pc["idx_src"], "idx_dst": pc["idx_dst"],
            "attr_t": pc["attr_t"],
        })
    return in_maps


def kernel(x, edge_index, edge_attr, W_l, W_r, W_e, att):
    global last_exec_time_ns, last_insts

    plan = _plan(edge_index)
    in_maps = _prep_inputs(x, edge_index, edge_attr, W_l, W_r, W_e, att, plan)

    key = plan["maxcnt"].tobytes()
    if key not in _CACHE:
        _CACHE[key] = _build_two_pass(plan)
    nc = _CACHE[key]

    try:
        res = run_bass_kernel_spmd(nc, in_maps, core_ids=list(range(NC)), trace=True)
        last_exec_time_ns = res.exec_time_ns
        last_insts = res.instructions_and_trace[0] if res.instructions_and_trace else None
    except Exception:
        res = run_bass_kernel_spmd(nc, in_maps, core_ids=list(range(NC)), trace=False)
        last_exec_time_ns = None
        last_insts = None

    return np.concatenate([res.results[c]["out"] for c in range(NC)], axis=0)


# revision 8
# speedup vs baseline: 2.7736x; 1.8353x over previous
"""GATv2Conv (heads=1, edge_dim=11, self-loops fill='mean') on 8 Trainium2 cores.

Dst-sharded (12500 own nodes/core). Per 128-dst chunk, segment reductions are
one-hot matmuls S^T @ [ex | e | ex*xl] accumulated in PSUM. The one-hot S is
GATHERED (fp8, exact 0/1) together with xr[dst] from a device-built dst-table
(rows [xr bf16 64 | onehot fp8 128] = 256B), and xl[src] is gathered from the
node table C[n] = [xl|xr] (bf16, 4 int16 row-banks). Gathers are batched per
~64-tile group (5 calls/group) to amortize the ~1us SWDGE fixed cost.

Self-loop attr term uses linearity: (sum attr)/deg @ W_e == (sum e)/deg, so e
is aggregated instead of attr and the epilogue needs no transposes.

Host work is layout only: sharding, sorting, index packing, dtype casts.
"""

import sys

sys.path.insert(0, "/opt/trn_rl_repo")

import numpy as np
import ml_dtypes

import concourse.bass as bass
import concourse.bacc as bacc
import concourse.tile as tile
import concourse.mybir as mybir
from concourse.bass_utils import run_bass_kernel_spmd

BF16 = ml_dtypes.bfloat16
AF = mybir.ActivationFunctionType
OP = mybir.AluOpType

N, E, DIN, DOUT, DE = 100000, 1000000, 128, 64, 11
NEG_SLOPE = 0.2
NC = 8
NOWN = N // NC                    # 12500
NCHUNK = 98                       # ceil(12500/128)
NPAD = NCHUNK * 128               # 12544
BANKS = 4
BANKROWS = 25088                  # 4*25088 = 100352 = XT_COLS; < 2^15 for int16
XT_COLS = BANKS * BANKROWS
DST_ROWS = NPAD + 1               # row 0 = zeros (pad target)
RHS_W = 129                       # [ex | e(64) | exxl(64)]
GROUP_TILES = 64                  # gather-group budget
GROUP_CHUNKS = 8                  # aggs slab depth
HGT = 32                          # half-group tiles (attr/rhs slab width)
SUB = 8                           # tiles per e-psum sub-block

last_exec_time_ns = None
last_insts = None
_CACHE = {}


def _cdiv(a, b):
    return -(-a // b)


def _bc3(ap2, mid):
    """[P, F] AP -> [P, mid, F] broadcast along a new middle dim."""
    return bass.AP(ap2.tensor, ap2.offset, [ap2.ap[0], [0, mid], ap2.ap[1]])


def _in3(ap2, inner):
    """[P, T] AP -> [P, T, inner] broadcast along a new inner dim."""
    return bass.AP(ap2.tensor, ap2.offset, [ap2.ap[0], ap2.ap[1], [0, inner]])


def _bcast_last(ap3, inner):
    """[P, T, 1] AP -> [P, T, inner] broadcast along the last dim."""
    return bass.AP(ap3.tensor, ap3.offset, [ap3.ap[0], ap3.ap[1], [0, inner]])


# --------------------------------------------------------------------------
# host-side layout (index manipulation only)
# --------------------------------------------------------------------------

def _plan(edge_index):
    src = np.asarray(edge_index[0]).astype(np.int64)
    dst = np.asarray(edge_index[1]).astype(np.int64)
    core = dst // NOWN
    ldst = dst - core * NOWN                      # 0..12499
    chunk = ldst >> 7                             # 0..97
    bank = src // BANKROWS                        # 0..3

    cellid = (core * NCHUNK + chunk) * BANKS + bank
    counts = np.bincount(cellid, minlength=NC * NCHUNK * BANKS)
    maxcnt = counts.reshape(NC, NCHUNK, BANKS).max(axis=0)     # [98, 4] shared
    ntiles_cb = (maxcnt + 127) // 128                          # [98, 4]
    t_ch = ntiles_cb.sum(axis=1)                               # [98]

    # group packing: consecutive chunks, sum(t_ch) <= GROUP_TILES, nch <= 8
    groups = []
    ch = 0
    while ch < NCHUNK:
        ch0, tl = ch, 0
        while (ch < NCHUNK and ch - ch0 < GROUP_CHUNKS
               and tl + t_ch[ch] <= GROUP_TILES):
            tl += int(t_ch[ch])
            ch += 1
        assert ch > ch0, f"chunk {ch0} too big: t_ch={t_ch[ch0]}"
        groups.append((ch0, ch - ch0, tl))

    # tile index assignment: group -> bank -> chunk
    cell_tile0 = np.zeros((NCHUNK, BANKS), np.int64)
    grp_meta = []
    tbase = 0
    for (ch0, nch, gt) in groups:
        g_tile0 = tbase
        bank_spans = []
        tile_chunk = []                    # chunk id per local tile
        for b in range(BANKS):
            b_t0 = tbase
            for c in range(ch0, ch0 + nch):
                cell_tile0[c, b] = tbase
                tbase += int(ntiles_cb[c, b])
                tile_chunk += [c] * int(ntiles_cb[c, b])
            bank_spans.append((b_t0 - g_tile0, tbase - b_t0))  # (local ofs, ntiles)
        # gather pieces (<= PIECE tiles), src/dst interleaved for queue overlap
        PIECE = 8
        sp, dp = [], []
        for b, (lofs, nt) in enumerate(bank_spans):
            for o in range(0, nt, PIECE):
                sp.append(("src", b, lofs + o, min(PIECE, nt - o)))
        for o in range(0, gt, PIECE):
            dp.append(("dst", None, o, min(PIECE, gt - o)))
        pieces = []
        i = j = 0
        while i < len(sp) or j < len(dp):
            if i < len(sp):
                pieces.append(sp[i]); i += 1
            if j < len(dp):
                pieces.append(dp[j]); j += 1
        grp_meta.append(dict(ch0=ch0, nch=nch, tile0=g_tile0, gt=gt,
                             bank_spans=bank_spans, tile_chunk=tile_chunk,
                             pieces=pieces))
        assert gt == tbase - g_tile0
    tot_tiles = tbase

    # per-edge slot: stable sort by cell start, rank within cell
    cell_start = cell_tile0[chunk, bank] * 128       # global slot base per edge
    order = np.argsort(core * (tot_tiles * 128) + cell_start, kind="stable")
    key_s = cell_start[order]
    core_s = core[order]
    # rank within (core, cell)
    csort = core_s * (tot_tiles * 128) + key_s
    starts = np.zeros(E, np.int64)
    newgrp = np.ones(E, bool)
    newgrp[1:] = csort[1:] != csort[:-1]
    idx_of_start = np.nonzero(newgrp)[0]
    grp_id = np.cumsum(newgrp) - 1
    rank = np.arange(E) - idx_of_start[grp_id]
    slot = key_s + rank                              # global slot per edge

    return dict(src_o=src[order], ldst_o=ldst[order], core_o=core_s,
                slot=slot, bank_o=bank[order], order=order,
                maxcnt=maxcnt, ntiles_cb=ntiles_cb, t_ch=t_ch,
                groups=grp_meta, tot_tiles=tot_tiles)


def _host_arrays(plan, edge_attr, dst):
    tot_tiles = plan["tot_tiles"]
    idxcols = tot_tiles * 8
    attr_o = np.asarray(edge_attr)[plan["order"]]
    per_core = []
    for c in range(NC):
        m = plan["core_o"] == c
        slot = plan["slot"][m]
        p16, c16 = slot % 16, slot // 16

        i_src = np.zeros((16, idxcols), np.int16)
        i_src[p16, c16] = (plan["src_o"][m] - plan["bank_o"][m] * BANKROWS
                           ).astype(np.int16)
        i_dst = np.zeros((16, idxcols), np.int16)
        i_dst[p16, c16] = (plan["ldst_o"][m] + 1).astype(np.int16)

        attr_t = np.zeros((DE, tot_tiles * 128), BF16)
        attr_t[:, slot] = attr_o[m].T.astype(BF16)

        deg = np.bincount(dst[(dst // NOWN) == c] - c * NOWN, minlength=NPAD)
        rdeg = (1.0 / np.maximum(deg, 1)).astype(np.float32)

        per_core.append(dict(idx_src=np.tile(i_src, (8, 1)),
                             idx_dst=np.tile(i_dst, (8, 1)),
                             attr_t=attr_t,
                             rdeg=rdeg.reshape(NCHUNK, 128).T.copy()))
    return per_core


def _onehot_fp8():
    oh = np.zeros((DST_ROWS, 128), np.uint8)
    j = np.arange(1, DST_ROWS)
    oh[j, (j - 1) & 127] = 0x38                      # fp8 e4m3 1.0
    return oh.view(BF16)                             # [DST_ROWS, 64] bf16-viewed


# --------------------------------------------------------------------------
# device program (one SPMD NEFF for 8 cores; layout baked from `plan`)
# --------------------------------------------------------------------------

def _build_device(plan, queue_map=None):
    """queue_map: list of queue_num per gather (emission order), or None (all 0).

    Tile assigns SWDGE completion-sem lanes (DMASW0-7) round-robin in
    SCHEDULED order; a lane must only ever be updated from one SWDGE queue
    or completion waits under-synchronize (a real HW race). So pass 1 builds
    with all gathers on queue 0, reads each gather's assigned lane, and pass 2
    rebuilds with queue = lane % 4 to get 4-way parallel descriptor gen.
    """
    dt = mybir.dt
    groups = plan["groups"]
    tot_tiles = plan["tot_tiles"]
    gathers = []

    nc = bacc.Bacc("TRN2", target_bir_lowering=False, debug=False,
                   num_devices=NC, num_swdge_queues=4)

    def gq():
        if queue_map is None:
            return 0
        return queue_map[len(gathers)]

    xT = nc.dram_tensor("xT", [128, XT_COLS], dt.bfloat16, kind="ExternalInput")
    x_ownT = nc.dram_tensor("x_ownT", [128, NPAD], dt.bfloat16, kind="ExternalInput")
    w_cat = nc.dram_tensor("w_cat", [128, 128], dt.bfloat16, kind="ExternalInput")
    wep = nc.dram_tensor("wep", [128, DOUT], dt.bfloat16, kind="ExternalInput")
    att_bc = nc.dram_tensor("att_bc", [128, DOUT], dt.bfloat16, kind="ExternalInput")
    oneh = nc.dram_tensor("oneh", [DST_ROWS, DOUT], dt.bfloat16, kind="ExternalInput")
    rdeg_d = nc.dram_tensor("rdeg", [128, NCHUNK], dt.float32, kind="ExternalInput")
    idx_src = nc.dram_tensor("idx_src", [128, tot_tiles * 8], dt.int16, kind="ExternalInput")
    idx_dst = nc.dram_tensor("idx_dst", [128, tot_tiles * 8], dt.int16, kind="ExternalInput")
    attr_t = nc.dram_tensor("attr_t", [DE, tot_tiles * 128], dt.bfloat16, kind="ExternalInput")
    out_d = nc.dram_tensor("out", [NOWN, DOUT], dt.float32, kind="ExternalOutput")

    qn = [0]

    def next_q():
        q = qn[0] & 3
        qn[0] += 1
        return q

    with tile.TileContext(nc) as tc:
        with (
            tc.tile_pool(name="const", bufs=1) as constp,
            tc.tile_pool(name="own", bufs=1) as ownp,
            tc.tile_pool(name="dram", bufs=1, space="DRAM") as dramp,
            tc.tile_pool(name="tload", bufs=2) as tloadp,
            tc.tile_pool(name="stage", bufs=2) as stagep,
            tc.tile_pool(name="gat", bufs=2) as gatp,
            tc.tile_pool(name="attr", bufs=2) as attrp,
            tc.tile_pool(name="rhs", bufs=2) as rhsp,
            tc.tile_pool(name="work", bufs=3) as workp,
            tc.tile_pool(name="agg", bufs=2) as aggp,
            tc.tile_pool(name="epi", bufs=2) as epip,
            tc.tile_pool(name="idx", bufs=2) as idxp,
            tc.tile_pool(name="psC", bufs=2, space="PSUM") as psC,
            tc.tile_pool(name="psE", bufs=2, space="PSUM") as psE,
            tc.tile_pool(name="psA", bufs=2, space="PSUM") as psA,
        ):
            # ---------------- constants
            wcat_b = constp.tile([128, 128], dt.bfloat16, tag="wcat_b")
            nc.sync.dma_start(wcat_b[:], w_cat[:])
            wep_b = constp.tile([128, DOUT], dt.bfloat16, tag="wep_b")
            nc.sync.dma_start(wep_b[:], wep[:])
            attb_b = constp.tile([128, DOUT], dt.bfloat16, tag="attb_b")
            nc.sync.dma_start(attb_b[:], att_bc[:])
            rdeg_sb = constp.tile([128, NCHUNK], dt.float32, tag="rdeg_sb")
            nc.sync.dma_start(rdeg_sb[:], rdeg_d[:])

            xl_own = ownp.tile([128, NCHUNK, DOUT], dt.bfloat16, tag="xl_own")
            xr_own = ownp.tile([128, NCHUNK, DOUT], dt.bfloat16, tag="xr_own")

            C_b = [dramp.tile([BANKROWS, 128], dt.bfloat16, name=f"C_b{b}")
                   for b in range(BANKS)]
            dst_tab = dramp.tile([DST_ROWS, 128], dt.bfloat16)



            # ---------------- phase B: own nodes, dst-table
            zrow = constp.tile([1, 128], dt.bfloat16, tag="zrow")
            nc.vector.memset(zrow[:], 0.0)
            nc.sync.dma_start(dst_tab[0:1, :], zrow[:])
            nc.sync.dma_start(dst_tab[1:1 + NPAD, DOUT:128],
                              oneh[1:1 + NPAD, :])
            for q in range(25):                       # quads of 4 chunks (98 = 24*4+2)
                nq = 4 if q < 24 else 2
                ch0 = q * 4
                xo = tloadp.tile([128, 4, 128], dt.bfloat16, tag="xo")
                nc.sync.dma_start(
                    xo[:, 0:nq, :],
                    x_ownT[:, ch0 * 128:(ch0 + nq) * 128].rearrange(
                        "p (t c) -> p t c", c=128))
                ps = psC.tile([128, 512], dt.float32, tag="psC")
                for k in range(nq):
                    nc.tensor.matmul(ps[:, k * 128:(k + 1) * 128],
                                     lhsT=xo[:, k, :], rhs=wcat_b[:],
                                     start=True, stop=True)
                ps3 = ps[:, 0:nq * 128].rearrange("p (t c) -> p t c", c=128)
                nc.vector.tensor_copy(xl_own[:, ch0:ch0 + nq, :], ps3[:, :, 0:DOUT])
                nc.scalar.copy(xr_own[:, ch0:ch0 + nq, :], ps3[:, :, DOUT:128])
            nc.sync.dma_start(
                dst_tab[1:1 + NPAD, 0:DOUT].rearrange("(c p) d -> p c d", p=128),
                xr_own[:, :, :])

            # ---------------- phase A: node table C[n] = [xl | xr] per bank
            NT_IT = 14                                # tiles per iteration
            for b in range(BANKS):
                for it in range(BANKROWS // (NT_IT * 128)):   # 14 iters
                    c0 = b * BANKROWS + it * NT_IT * 128
                    xt = tloadp.tile([128, NT_IT, 128], dt.bfloat16, tag="xt")
                    nc.sync.dma_start(
                        xt[:, :, :],
                        xT[:, c0:c0 + NT_IT * 128].rearrange(
                            "p (t c) -> p t c", c=128))
                    stg = stagep.tile([128, NT_IT, 128], dt.bfloat16, tag="stg")
                    for q in range(NT_IT // 2):       # 7 psum pairs of 2 tiles
                        ps = psC.tile([128, 512], dt.float32, tag="psC")
                        for k in range(2):
                            nc.tensor.matmul(ps[:, k * 128:(k + 1) * 128],
                                             lhsT=xt[:, q * 2 + k, :],
                                             rhs=wcat_b[:], start=True, stop=True)
                        eng = nc.vector if (q & 1) else nc.scalar
                        if q & 1:
                            nc.vector.tensor_copy(
                                stg[:, q * 2:q * 2 + 2, :],
                                ps[:, 0:256].rearrange("p (t c) -> p t c", c=128))
                        else:
                            nc.scalar.copy(
                                stg[:, q * 2:q * 2 + 2, :],
                                ps[:, 0:256].rearrange("p (t c) -> p t c", c=128))
                    nc.sync.dma_start(
                        C_b[b][it * NT_IT * 128:(it + 1) * NT_IT * 128, :]
                        .rearrange("(t p) c -> p t c", p=128),
                        stg[:, :, :])

            # ---------------- phase C: per-group edge pipeline
            for g in groups:
                ch0, nch, t0, gt = g["ch0"], g["nch"], g["tile0"], g["gt"]
                tile_chunk = g["tile_chunk"]

                isrc = idxp.tile([128, GROUP_TILES * 8], dt.int16, tag="isrc")
                nc.sync.dma_start(isrc[:, 0:gt * 8],
                                  idx_src[:, t0 * 8:(t0 + gt) * 8])
                idst = idxp.tile([128, GROUP_TILES * 8], dt.int16, tag="idst")
                nc.sync.dma_start(idst[:, 0:gt * 8],
                                  idx_dst[:, t0 * 8:(t0 + gt) * 8])

                g_src = gatp.tile([128, GROUP_TILES, 128], dt.bfloat16, tag="g_src")
                g_dst = gatp.tile([128, GROUP_TILES, 128], dt.bfloat16, tag="g_dst")
                for (kind, b, lofs, nt) in g["pieces"]:
                    if kind == "src":
                        out_ap, in_ap, idx = g_src, C_b[b][:, :], isrc
                    else:
                        out_ap, in_ap, idx = g_dst, dst_tab[:, :], idst
                    gi = nc.gpsimd.dma_gather(
                        out_ap=out_ap[:, lofs:lofs + nt, :], in_ap=in_ap,
                        idxs_ap=idx[:, lofs * 8:(lofs + nt) * 8],
                        num_idxs=nt * 128, num_idxs_reg=nt * 128,
                        elem_size=128, queue_num=gq(), single_packet=False)
                    gathers.append((gi, nt * 128))

                aggs = aggp.tile([128, GROUP_CHUNKS, RHS_W], dt.float32, tag="aggs")
                seen_chunk = set()

                nhalf = _cdiv(gt, HGT)
                for h in range(nhalf):
                    h0 = h * HGT
                    hn = min(HGT, gt - h0)
                    apad = attrp.tile([DE, HGT, 128], dt.bfloat16, tag="attrpad")
                    nc.sync.dma_start(
                        apad[:, 0:hn, :],
                        attr_t[:, (t0 + h0) * 128:(t0 + h0 + hn) * 128]
                        .rearrange("a (t c) -> a t c", c=128))
                    rhs = rhsp.tile([128, HGT, RHS_W], dt.bfloat16, tag="rhs")

                    for s0 in range(0, hn, SUB):
                        ns = min(SUB, hn - s0)
                        ts = h0 + s0                      # group-local tile base
                        pse = psE.tile([128, SUB * DOUT], dt.float32, tag="psE")
                        for i in range(ns):
                            nc.tensor.matmul(pse[:, i * DOUT:(i + 1) * DOUT],
                                             lhsT=apad[:, s0 + i, :],
                                             rhs=wep_b[0:DE, :],
                                             start=True, stop=True)
                        pse3 = pse[:, 0:ns * DOUT].rearrange("p (t d) -> p t d", d=DOUT)
                        nc.scalar.copy(rhs[:, s0:s0 + ns, 1:1 + DOUT], pse3)

                        m1 = workp.tile([128, SUB, DOUT], dt.bfloat16, tag="m1")
                        nc.vector.tensor_tensor(
                            out=m1[:, 0:ns, :], in0=g_src[:, ts:ts + ns, 0:DOUT],
                            in1=g_dst[:, ts:ts + ns, 0:DOUT], op=OP.add)
                        nc.vector.tensor_tensor(
                            out=m1[:, 0:ns, :], in0=m1[:, 0:ns, :],
                            in1=rhs[:, s0:s0 + ns, 1:1 + DOUT], op=OP.add)
                        nc.scalar.activation(m1[:, 0:ns, :], m1[:, 0:ns, :],
                                             AF.Prelu, alpha=NEG_SLOPE)
                        lt = workp.tile([128, SUB, DOUT], dt.bfloat16, tag="lt")
                        nc.vector.tensor_tensor(out=lt[:, 0:ns, :], in0=m1[:, 0:ns, :],
                                                in1=_bc3(attb_b[:, :], ns), op=OP.mult)
                        lg = workp.tile([128, SUB], dt.float32, tag="lg")
                        nc.vector.tensor_reduce(out=lg[:, 0:ns], in_=lt[:, 0:ns, :],
                                                axis=mybir.AxisListType.X, op=OP.add)
                        nc.scalar.activation(
                            rhs[:, s0:s0 + ns, 0:1],
                            lg[:, 0:ns].rearrange("p (t o) -> p t o", o=1), AF.Exp)
                        nc.vector.tensor_tensor(
                            out=rhs[:, s0:s0 + ns, 1 + DOUT:RHS_W],
                            in0=g_src[:, ts:ts + ns, 0:DOUT],
                            in1=_bcast_last(rhs[:, s0:s0 + ns, 0:1], DOUT),
                            op=OP.mult)

                        # agg spans (runs of equal chunk within this sub-block)
                        i = 0
                        while i < ns:
                            j = i
                            cch = tile_chunk[ts + i]
                            while j < ns and tile_chunk[ts + j] == cch:
                                j += 1
                            pa = psA.tile([128, RHS_W], dt.float32, tag="psA")
                            for t in range(i, j):
                                nc.tensor.matmul(
                                    pa[:],
                                    lhsT=g_dst[:, ts + t, DOUT:128].bitcast(dt.float8e4),
                                    rhs=rhs[:, s0 + t, :],
                                    start=(t == i), stop=(t == j - 1))
                            cl = cch - ch0
                            if cch in seen_chunk:
                                nc.vector.tensor_tensor(out=aggs[:, cl, :],
                                                        in0=aggs[:, cl, :],
                                                        in1=pa[:], op=OP.add)
                            else:
                                seen_chunk.add(cch)
                                nc.vector.tensor_copy(aggs[:, cl, :], pa[:])
                            i = j

                # ---------- per-group epilogue (self-loop + normalize + store)
                lep = epip.tile([128, GROUP_CHUNKS, DOUT], dt.float32, tag="lep")
                nc.vector.tensor_tensor(out=lep[:, 0:nch, :],
                                        in0=aggs[:, 0:nch, 1:1 + DOUT],
                                        in1=_in3(rdeg_sb[:, ch0:ch0 + nch], DOUT),
                                        op=OP.mult)
                nc.vector.tensor_tensor(out=lep[:, 0:nch, :], in0=lep[:, 0:nch, :],
                                        in1=xl_own[:, ch0:ch0 + nch, :], op=OP.add)
                nc.vector.tensor_tensor(out=lep[:, 0:nch, :], in0=lep[:, 0:nch, :],
                                        in1=xr_own[:, ch0:ch0 + nch, :], op=OP.add)
                mlb = epip.tile([128, GROUP_CHUNKS, DOUT], dt.bfloat16, tag="mlb")
                nc.scalar.activation(mlb[:, 0:nch, :], lep[:, 0:nch, :],
                                     AF.Prelu, alpha=NEG_SLOPE)
                nc.vector.tensor_tensor(out=mlb[:, 0:nch, :], in0=mlb[:, 0:nch, :],
                                        in1=_bc3(attb_b[:, :], nch), op=OP.mult)
                exl = epip.tile([128, GROUP_CHUNKS], dt.float32, tag="exl")
                nc.vector.tensor_reduce(out=exl[:, 0:nch], in_=mlb[:, 0:nch, :],
                                        axis=mybir.AxisListType.X, op=OP.add)
                nc.scalar.activation(exl[:, 0:nch], exl[:, 0:nch], AF.Exp)
                rden = epip.tile([128, GROUP_CHUNKS], dt.float32, tag="rden")
                nc.vector.tensor_tensor(out=rden[:, 0:nch], in0=aggs[:, 0:nch, 0],
                                        in1=exl[:, 0:nch], op=OP.add)
                nc.vector.reciprocal(rden[:, 0:nch], rden[:, 0:nch])
                o = epip.tile([128, GROUP_CHUNKS, DOUT], dt.float32, tag="o")
                nc.vector.tensor_tensor(out=o[:, 0:nch, :],
                                        in0=xl_own[:, ch0:ch0 + nch, :],
                                        in1=_in3(exl[:, 0:nch], DOUT), op=OP.mult)
                nc.vector.tensor_tensor(out=o[:, 0:nch, :], in0=o[:, 0:nch, :],
                                        in1=aggs[:, 0:nch, 1 + DOUT:RHS_W], op=OP.add)
                nc.vector.tensor_tensor(out=o[:, 0:nch, :], in0=o[:, 0:nch, :],
                                        in1=_in3(rden[:, 0:nch], DOUT), op=OP.mult)

                nfull = nch if (ch0 + nch) * 128 <= NOWN else nch - 1
                if nfull > 0:
                    nc.sync.dma_start(
                        out_d[ch0 * 128:(ch0 + nfull) * 128, :]
                        .rearrange("(c p) d -> p c d", p=128),
                        o[:, 0:nfull, :])
                if nfull < nch:
                    rows = NOWN - (ch0 + nfull) * 128
                    nc.sync.dma_start(
                        out_d[(ch0 + nfull) * 128:NOWN, :],
                        o[0:rows, nfull, :])

    nc.compile()
    return nc, gathers


def _gather_lanes(gathers):
    """Read the scheduler-assigned DMASW lane for each gather."""
    from concourse.tile_scheduler import PROC_NAMES
    lanes = []
    for gi, _rows in gathers:
        proc = getattr(gi.ins, "bass_scheduled_proc", None)
        name = PROC_NAMES[proc] if proc is not None else None
        assert name is not None and name.startswith("DMASW"), (proc, name)
        lanes.append(int(name[5:]))
    return lanes


def _assign_queues(gathers, lanes):
    """lane -> queue, balancing descriptor-gen rows. Queue 0 descgen runs
    synchronously on the GpSimd sequencer (blocks issue), queues 1-3 run on
    other Q7 cores; weight q0 as more expensive."""
    lane_rows = [0] * 8
    for (_gi, rows), ln in zip(gathers, lanes):
        lane_rows[ln] += rows
    order = sorted(range(8), key=lambda ln: -lane_rows[ln])
    load = [0.0, 0.0, 0.0, 0.0]
    weight = [1.6, 1.0, 1.0, 1.0]
    lane_q = [0] * 8
    for ln in order:
        q = min(range(4), key=lambda qq: (load[qq] + lane_rows[ln]) * weight[qq])
        lane_q[ln] = q
        load[q] += lane_rows[ln]
    return lane_q


def _build_two_pass(plan):
    nc1, gathers1 = _build_device(plan)
    lanes = _gather_lanes(gathers1)
    lane_q = _assign_queues(gathers1, lanes)
    qmap = [lane_q[ln] for ln in lanes]
    nc2, gathers2 = _build_device(plan, queue_map=qmap)
    lanes2 = _gather_lanes(gathers2)
    if lanes2 != lanes:                       # schedule shifted: fall back safe
        return nc1
    return nc2


# --------------------------------------------------------------------------
# entry point
# --------------------------------------------------------------------------

def _prep_inputs(x, edge_index, edge_attr, W_l, W_r, W_e, att, plan):
    per_core = _host_arrays(plan, np.asarray(edge_attr, np.float32),
                            np.asarray(edge_index[1]).astype(np.int64))

    x = np.asarray(x, np.float32)
    xT = np.zeros((128, XT_COLS), BF16)
    xT[:, :N] = x.T.astype(BF16)
    w_cat = np.concatenate([np.asarray(W_l, np.float32),
                            np.asarray(W_r, np.float32)], axis=1).astype(BF16)
    wep = np.zeros((128, DOUT), BF16)
    wep[:DE] = np.asarray(W_e, np.float32).astype(BF16)
    att_bc = np.tile(np.asarray(att, np.float32)[None, :], (128, 1)).astype(BF16)
    oneh = _onehot_fp8()

    in_maps = []
    for c in range(NC):
        x_ownT = np.zeros((128, NPAD), BF16)
        x_ownT[:, :NOWN] = x[c * NOWN:(c + 1) * NOWN].T.astype(BF16)
        pc = per_core[c]
        in_maps.append({
            "xT": xT, "x_ownT": x_ownT, "w_cat": w_cat, "wep": wep,
            "att_bc": att_bc, "oneh": oneh, "rdeg": pc["rdeg"],
            "idx_src": pc["idx_src"], "idx_dst": pc["idx_dst"],
            "attr_t": pc["attr_t"],
        })
    return in_maps


def kernel(x, edge_index, edge_attr, W_l, W_r, W_e, att):
    global last_exec_time_ns, last_insts

    plan = _plan(edge_index)
    in_maps = _prep_inputs(x, edge_index, edge_attr, W_l, W_r, W_e, att, plan)

    key = plan["maxcnt"].tobytes()
    if key not in _CACHE:
        _CACHE[key] = _build_two_pass(plan)
    nc = _CACHE[key]

    try:
        res = run_bass_kernel_spmd(nc, in_maps, core_ids=list(range(NC)), trace=True)
        last_exec_time_ns = res.exec_time_ns
        last_insts = res.instructions_and_trace[0] if res.instructions_and_trace else None
    except Exception:
        res = run_bass_kernel_spmd(nc, in_maps, core_ids=list(range(NC)), trace=False)
        last_exec_time_ns = None
        last_insts = None

    return np.concatenate([res.results[c]["out"] for c in range(NC)], axis=0)
